# revision 44
# baseline (speedup 1.0000x reference)
# Trainium2 Bass kernel for EndPointRepr (span endpoint representations).
#
# reference:
#   h = encoded_input @ W + b                    # [B, S, P]
#   res_k[q] = concat(h[qb[q], s_k[q]], h[qb[q], e_k[q]]) * (e_k[q] >= s_k[q])
#
# Sharding: data-parallel over batch. Core c owns batch c; the host routes
# each query to its batch's core.
#
# Device pipeline (fp16 matmul, fp32 PSUM accumulation):
#   - host ships x^T (k-major) fp16, so matmul lhsT tiles come straight from
#     DMA: no PE transposes, no PSUM->SBUF transpose copies, and the X read
#     is half the fp32 bytes.
#   - h lives entirely in SBUF as [128, 16, 256] fp16 (h[m*128+p] at
#     partition p, rank m) - exactly the token layout SBUF-source dma_gather
#     wants (tokens_per_rank=128).
#   - endpoint gathers are SWDGE prepare_only ops: descriptor generation
#     (the ~7ns/idx slot scan, the dominant GpSimd cost) runs during the
#     matmul phase since it only needs the idx table; trigger_dma fires the
#     actual SBUF->SBUF gather on the DMA engines once h is in SBUF. One
#     combined gather per stream-pair ([s slots | e slots]) keeps the
#     per-prep fixed costs (ucode launch + Tile's InstIncSwdgeSem pre-bump,
#     ~2.5us each) to two.
#   - Tile's DMASW lane accounting for prepare_only preps is pre-credited
#     (the consumer waits it emits are vacuous), so the output writes are
#     gated on the descriptor-baked DMA sems via explicit ACT wait_ge.
#   - gathered tiles are [128, 2, 2*cap] (transposed), dumped to transposed
#     compact outputs r1t/r2t [256, 2*cap] fp16; the host de-transposes,
#     upcasts, adds bias, and scatters into the full [NQ, 512] fp32 results.
#     Invalid queries (e < s) are never routed; their rows stay zero.
# Slot capacities are computed from the actual inputs at build time (the
# program is compiled after kernel() sees its arguments) and the build is
# cached on the capacity.
import numpy as np

B, S, D, P = 8, 2048, 1024, 256
NQ = 8192
NCORES = 8
KB = D // 128           # contraction k-blocks
MB = S // 128           # h row-blocks (SBUF ranks)
NCH = 8                 # x column chunks (pipelines DMA under matmul)
CHW = S // NCH          # rows of x per chunk

_cache = {}


MAXG = 896              # HW limit: transpose gathers crash above ~1024 idx


def _gather_sizes(total):
    """Split `total` slots (multiple of 128) into chunks of <=MAXG, each a
    multiple of 128. The leading 128-slot chunk exists to pay the GpSimd
    ucode library-load (~10-15us, hidden inside the first SWDGE op) as
    early as possible."""
    sizes = []
    rem = total
    while rem > MAXG:
        sizes.append(768)
        rem -= 768
    sizes.append(rem)
    return sizes


def _build_nc(cap):
    import concourse.bacc as bacc
    import concourse.mybir as mybir
    import concourse.tile as tile
    from concourse.tile import add_dep_helper

    f16 = mybir.dt.float16
    f32 = mybir.dt.float32
    TOT = 4 * cap           # global slots: s1 | e1 | s2 | e2
    sizes = _gather_sizes(TOT)
    offs = [sum(sizes[:i]) for i in range(len(sizes))]

    nc = bacc.Bacc("TRN2", target_bir_lowering=False, debug=False,
                   num_devices=NCORES, num_swdge_queues=1)

    x = nc.dram_tensor("x", [NCH, 128, KB, CHW], f16,
                       kind="ExternalInput").ap()
    w = nc.dram_tensor("w", [128, KB, P], f16, kind="ExternalInput").ap()
    idx = nc.dram_tensor("idx", [128, TOT // 16], mybir.dt.int16,
                         kind="ExternalInput").ap()
    r1t = nc.dram_tensor("r1t", [P, 2 * cap], f16, kind="ExternalOutput").ap()
    r2t = nc.dram_tensor("r2t", [P, 2 * cap], f16, kind="ExternalOutput").ap()
    hdbg = None
    if _cache.get("debug_h"):
        hdbg = nc.dram_tensor("hdbg", [128, MB, P], f16,
                              kind="ExternalOutput").ap()

    h_store = nc.alloc_sbuf_tensor("hstore", [128, MB, P], f16)
    idx_store = None

    with tile.TileContext(nc) as tc:
        with (
            tc.tile_pool(name="consts", bufs=1) as consts,
            tc.tile_pool(name="xch", bufs=NCH) as xch_pool,
            tc.tile_pool(name="gath", bufs=1) as g_pool,
            tc.tile_pool(name="psh", bufs=4, space="PSUM") as psum_pool,
        ):
            from concourse import library_config
            li = nc.gpsimd.load_library(library_config.mlp)
            li.ins.bass_priority = -30000
            idx_sb = nc.alloc_sbuf_tensor(
                "idxstore", [128, TOT // 16], mybir.dt.int16).ap()
            ild = nc.scalar.dma_start(idx_sb, idx)
            ild.ins.bass_priority = -20000
            w_sb = consts.tile([128, KB, P], f16)
            wld = nc.sync.dma_start(w_sb, w)
            h_sb = h_store.ap()
            h_gather = h_sb

            # gather preps first: they only need idx_sb, so the slot scan
            # (the dominant GpSimd cost, ~9ns/idx) runs under the matmuls.
            g_tiles, g_sems, preps = [], [], []
            for gi, (n, off) in enumerate(zip(sizes, offs)):
                sem = nc.alloc_semaphore(f"gsem{gi}")
                g_sb = g_pool.tile([128, 2, n], f16, tag=f"g{gi}",
                                   name=f"g{gi}")
                g_tiles.append(g_sb)
                g_sems.append(sem)
                pi = nc.gpsimd.dma_gather(
                    g_sb, h_gather, idx_sb[:, off // 16:(off + n) // 16],
                    num_idxs=n, num_idxs_reg=n, elem_size=P,
                    transpose=True, prepare_only=True, sem=sem,
                    queue_num=0, sbuf_tokens_per_rank=128,
                    sbuf_free_dim_per_rank=2 * P)
                add_dep_helper(pi.ins, ild.ins, reason="prep reads idx")
                pi.ins.bass_priority = -10000 + gi
                preps.append(pi)

            xc, xlds = [], []
            for c in range(NCH):
                t = xch_pool.tile([128, KB, CHW], f16, tag="xc")
                xlds.append(nc.sync.dma_start(t, x[c]))
                xc.append(t)

            # h = x @ W, one [128, P] row-block per rank of h_sb (bias is
            # added host-side; spec fills it with zeros anyway)
            copies = []
            mpc = MB // NCH
            for m in range(MB):
                c, off = m // mpc, (m % mpc) * 128
                h_ps = psum_pool.tile([128, P], f32, tag="hps")
                for kb in range(KB):
                    nc.tensor.matmul(h_ps, xc[c][:, kb, off:off + 128],
                                     w_sb[:, kb, :],
                                     start=(kb == 0), stop=(kb == KB - 1))
                ci = nc.vector.tensor_copy(h_sb[:, m, :], h_ps)
                copies.append(ci.ins)
            if hdbg is not None:
                nc.scalar.dma_start(hdbg, h_sb)

            # two triggers on one queue: gathers 0-1 fire once h and their
            # scans are done, so their transfers+writes overlap the
            # remaining prep scans (ring append-while-consume is safe on a
            # single queue; concurrent queues are not).
            trig = nc.gpsimd.trigger_dma(count=None, queue_num=0)
            for ci in copies:
                add_dep_helper(trig.ins, ci, reason="gather reads h")
            add_dep_helper(trig.ins, preps[-1].ins, sync=False,
                           reason="trigger after prep scans")
            views = [r1t.rearrange("(q p) s -> p q s", p=128),
                     r2t.rearrange("(q p) s -> p q s", p=128)]

            def emit_writes(gi, n, off):
                weng = nc.scalar if gi % 2 == 0 else nc.sync
                wg = weng.wait_ge(g_sems[gi], 16)
                add_dep_helper(wg.ins, ild.ins, sync=False,
                               reason="queue issues idx load before blocking")
                add_dep_helper(wg.ins, wld.ins, sync=False,
                               reason="queue issues w load before blocking")
                for xl in xlds:
                    add_dep_helper(wg.ins, xl.ins, sync=False,
                                   reason="queue issues x loads first")
                add_dep_helper(wg.ins, trig.ins, sync=False,
                               reason="wait follows trigger")
                last = None
                lo = off
                while lo < off + n:
                    pair = lo // (2 * cap)
                    hi = min(off + n, (pair + 1) * 2 * cap)
                    last = weng.dma_start(
                        views[pair][:, :, lo - pair * 2 * cap:
                                    hi - pair * 2 * cap],
                        g_tiles[gi][:, :, lo - off:hi - off])
                    add_dep_helper(last.ins, wg.ins,
                                   reason="write after gather DMA sem")
                    lo = hi
                return last

            for gi in range(len(sizes)):
                emit_writes(gi, sizes[gi], offs[gi])

    nc.compile()
    return nc


def _get_nc(cap):
    key = ("nc", cap)
    if key not in _cache:
        _cache[key] = _build_nc(cap)
    return _cache[key]


def _numpy_ref(flag, encoded_input, start_ids_1, end_ids_1, query_batch_idx,
               start_ids_2, end_ids_2, W, b):
    h = encoded_input.astype(np.float32) @ W.astype(np.float32) + \
        b.astype(np.float32)
    qb = np.asarray(query_batch_idx).astype(np.int64)

    def span(s, e):
        s = np.asarray(s).astype(np.int64)
        e = np.asarray(e).astype(np.int64)
        rep = np.concatenate([h[qb, s], h[qb, e]], axis=-1)
        return rep * (e >= s)[:, None].astype(rep.dtype)

    return span(start_ids_1, end_ids_1), span(start_ids_2, end_ids_2)


def _route_pair(s, e, sel, cap):
    """Pack one stream-pair's valid queries (global ids `sel`) into cap
    slots. Returns (idx_s, idx_e, order): cap-long slot->row maps (pads
    point at row 0) and slot->query-id (-1 = pad, dropped by the host)."""
    sv, ev = s[sel], e[sel]
    valid = ev >= sv
    pos = np.nonzero(valid)[0]
    if len(pos) > cap:
        raise ValueError("slot overflow")
    idx_s = np.zeros(cap, np.int64)
    idx_e = np.zeros(cap, np.int64)
    order = np.full(cap, -1, np.int64)
    idx_s[:len(pos)] = sv[pos]
    idx_e[:len(pos)] = ev[pos]
    order[:len(pos)] = sel[pos]
    return idx_s, idx_e, order


def kernel(flag, encoded_input, start_ids_1, end_ids_1, query_batch_idx,
           start_ids_2, end_ids_2, W, b):
    from concourse.bass_utils import run_bass_kernel_spmd

    x_full = np.asarray(encoded_input)
    w_np = np.asarray(W).astype(np.float32)
    b_np = np.asarray(b).astype(np.float32)
    qb = np.asarray(query_batch_idx).astype(np.int64)
    s1 = np.asarray(start_ids_1).astype(np.int64)
    e1 = np.asarray(end_ids_1).astype(np.int64)
    s2 = np.asarray(start_ids_2).astype(np.int64)
    e2 = np.asarray(end_ids_2).astype(np.int64)

    perms = [np.nonzero(qb == bb)[0] for bb in range(B)]
    in_range = (qb.min() >= 0 and qb.max() < B and
                all(a.min() >= 0 and a.max() < S for a in (s1, e1, s2, e2)))
    if not in_range or x_full.shape != (B, S, D):
        res1, res2 = _numpy_ref(flag, x_full, s1, e1, qb, s2, e2, w_np, b_np)
        return np.asarray(res1, np.float32), np.asarray(res2, np.float32)

    # per-(core, pair) valid counts -> slot capacity (64-granular so the
    # combined [s|e] gather length stays a multiple of 128)
    cmax = 1
    for bb in range(B):
        sel = perms[bb]
        for s, e in ((s1, e1), (s2, e2)):
            cmax = max(cmax, int(np.count_nonzero(e[sel] >= s[sel])))
    cap = -(-cmax // 64) * 64

    w16 = np.ascontiguousarray(
        w_np.reshape(KB, 128, P).transpose(1, 0, 2).astype(np.float16))

    in_maps, orders = [], []
    for bb in range(B):
        sel = perms[bb]
        i1s, i1e, order1 = _route_pair(s1, e1, sel, cap)
        i2s, i2e, order2 = _route_pair(s2, e2, sel, cap)
        orders.append((order1, order2))
        idx_flat = np.concatenate([i1s, i1e, i2s, i2e]).astype(np.int16)
        idx_w = idx_flat.reshape(4 * cap // 16, 16).T
        idx_w = np.ascontiguousarray(np.tile(idx_w, (8, 1)))
        # x chunk layout [c, p, k, j]: x16[c, p, k, j] = x[bb][c*CHW+j, k*128+p]
        x16 = np.ascontiguousarray(
            np.asarray(x_full[bb], np.float32)
            .reshape(NCH, CHW, KB, 128).transpose(0, 3, 2, 1)
            .astype(np.float16))
        in_maps.append({
            "x": x16,
            "w": w16,
            "idx": idx_w,
        })

    nc = _get_nc(cap)
    out = run_bass_kernel_spmd(nc, in_maps, core_ids=list(range(NCORES)))
    _cache["last_run"] = out

    bcat = np.concatenate([b_np, b_np]).astype(np.float32)
    res1 = np.zeros((NQ, 2 * P), np.float32)
    res2 = np.zeros((NQ, 2 * P), np.float32)
    for bb in range(B):
        order1, order2 = orders[bb]
        for r, order, res in ((out.results[bb]["r1t"], order1, res1),
                              (out.results[bb]["r2t"], order2, res2)):
            rt = r.T                      # [2*cap, 512]
            real = order >= 0
            vals = np.concatenate([rt[:cap][real], rt[cap:][real]],
                                  axis=1).astype(np.float32)
            res[order[real]] = vals + bcat
    return res1, res2


# revision 45
# speedup vs baseline: 1.1189x; 1.1189x over previous
# Trainium2 Bass kernel for EndPointRepr (span endpoint representations).
#
# reference:
#   h = encoded_input @ W + b                    # [B, S, P]
#   res_k[q] = concat(h[qb[q], s_k[q]], h[qb[q], e_k[q]]) * (e_k[q] >= s_k[q])
#
# Sharding: data-parallel over batch. Core c owns batch c; the host routes
# each query to its batch's core.
#
# Device pipeline (fp16 matmul, fp32 PSUM accumulation):
#   - host ships x^T (k-major) fp16, so matmul lhsT tiles come straight from
#     DMA: no PE transposes, no PSUM->SBUF transpose copies, and the X read
#     is half the fp32 bytes.
#   - h lives entirely in SBUF as [128, 16, 256] fp16 (h[m*128+p] at
#     partition p, rank m) - exactly the token layout SBUF-source dma_gather
#     wants (tokens_per_rank=128).
#   - endpoint gathers are SWDGE prepare_only ops: descriptor generation
#     (the ~7ns/idx slot scan, the dominant GpSimd cost) runs during the
#     matmul phase since it only needs the idx table; trigger_dma fires the
#     actual SBUF->SBUF gather on the DMA engines once h is in SBUF. One
#     combined gather per stream-pair ([s slots | e slots]) keeps the
#     per-prep fixed costs (ucode launch + Tile's InstIncSwdgeSem pre-bump,
#     ~2.5us each) to two.
#   - Tile's DMASW lane accounting for prepare_only preps is pre-credited
#     (the consumer waits it emits are vacuous), so the output writes are
#     gated on the descriptor-baked DMA sems via explicit ACT wait_ge.
#   - gathered tiles are [128, 2, 2*cap] (transposed), dumped to transposed
#     compact outputs r1t/r2t [256, 2*cap] fp16; the host de-transposes,
#     upcasts, adds bias, and scatters into the full [NQ, 512] fp32 results.
#     Invalid queries (e < s) are never routed; their rows stay zero.
# Slot capacities are computed from the actual inputs at build time (the
# program is compiled after kernel() sees its arguments) and the build is
# cached on the capacity.
import numpy as np

B, S, D, P = 8, 2048, 1024, 256
NQ = 8192
NCORES = 8
KB = D // 128           # contraction k-blocks
MB = S // 128           # h row-blocks (SBUF ranks)
NCH = 8                 # x column chunks (pipelines DMA under matmul)
CHW = S // NCH          # rows of x per chunk

_cache = {}


MAXG = 896              # HW limit: transpose gathers crash above ~1024 idx


def _gather_sizes(total):
    """Split `total` slots (multiple of 128) into chunks of <=MAXG, each a
    multiple of 128. The leading 128-slot chunk exists to pay the GpSimd
    ucode library-load (~10-15us, hidden inside the first SWDGE op) as
    early as possible."""
    sizes = [128]
    rem = total - 128
    while rem > MAXG:
        sizes.append(MAXG)
        rem -= MAXG
    sizes.append(rem)
    return sizes


def _build_nc(cap):
    import concourse.bacc as bacc
    import concourse.mybir as mybir
    import concourse.tile as tile
    from concourse.tile import add_dep_helper

    f16 = mybir.dt.float16
    f32 = mybir.dt.float32
    TOT = 4 * cap           # global slots: s1 | e1 | s2 | e2
    sizes = _gather_sizes(TOT)
    offs = [sum(sizes[:i]) for i in range(len(sizes))]

    nc = bacc.Bacc("TRN2", target_bir_lowering=False, debug=False,
                   num_devices=NCORES, num_swdge_queues=1)

    x = nc.dram_tensor("x", [NCH, 128, KB, CHW], f16,
                       kind="ExternalInput").ap()
    w = nc.dram_tensor("w", [128, KB, P], f16, kind="ExternalInput").ap()
    idx = nc.dram_tensor("idx", [128, TOT // 16], mybir.dt.int16,
                         kind="ExternalInput").ap()
    r1t = nc.dram_tensor("r1t", [P, 2 * cap], f16, kind="ExternalOutput").ap()
    r2t = nc.dram_tensor("r2t", [P, 2 * cap], f16, kind="ExternalOutput").ap()
    hdbg = None
    if _cache.get("debug_h"):
        hdbg = nc.dram_tensor("hdbg", [128, MB, P], f16,
                              kind="ExternalOutput").ap()

    h_store = nc.alloc_sbuf_tensor("hstore", [128, MB, P], f16)
    idx_store = None

    with tile.TileContext(nc) as tc:
        with (
            tc.tile_pool(name="consts", bufs=1) as consts,
            tc.tile_pool(name="xch", bufs=NCH) as xch_pool,
            tc.tile_pool(name="gath", bufs=1) as g_pool,
            tc.tile_pool(name="psh", bufs=4, space="PSUM") as psum_pool,
        ):
            from concourse import library_config
            li = nc.gpsimd.load_library(library_config.mlp)
            li.ins.bass_priority = -30000
            idx_sb = nc.alloc_sbuf_tensor(
                "idxstore", [128, TOT // 16], mybir.dt.int16).ap()
            ild = nc.scalar.dma_start(idx_sb, idx)
            ild.ins.bass_priority = -20000
            w_sb = consts.tile([128, KB, P], f16)
            wld = nc.sync.dma_start(w_sb, w)
            h_sb = h_store.ap()
            h_gather = h_sb

            # gather preps first: they only need idx_sb, so the slot scan
            # (the dominant GpSimd cost, ~9ns/idx) runs under the matmuls.
            g_tiles, g_sems, preps = [], [], []
            for gi, (n, off) in enumerate(zip(sizes, offs)):
                sem = nc.alloc_semaphore(f"gsem{gi}")
                g_sb = g_pool.tile([128, 2, n], f16, tag=f"g{gi}",
                                   name=f"g{gi}")
                g_tiles.append(g_sb)
                g_sems.append(sem)
                pi = nc.gpsimd.dma_gather(
                    g_sb, h_gather, idx_sb[:, off // 16:(off + n) // 16],
                    num_idxs=n, num_idxs_reg=n, elem_size=P,
                    transpose=True, prepare_only=True, sem=sem,
                    queue_num=0, sbuf_tokens_per_rank=128,
                    sbuf_free_dim_per_rank=2 * P)
                add_dep_helper(pi.ins, ild.ins, reason="prep reads idx")
                pi.ins.bass_priority = -10000 + gi
                preps.append(pi)

            xc, xlds = [], []
            for c in range(NCH):
                t = xch_pool.tile([128, KB, CHW], f16, tag="xc")
                xlds.append(nc.sync.dma_start(t, x[c]))
                xc.append(t)

            # h = x @ W, one [128, P] row-block per rank of h_sb (bias is
            # added host-side; spec fills it with zeros anyway)
            copies = []
            mpc = MB // NCH
            for m in range(MB):
                c, off = m // mpc, (m % mpc) * 128
                h_ps = psum_pool.tile([128, P], f32, tag="hps")
                for kb in range(KB):
                    nc.tensor.matmul(h_ps, xc[c][:, kb, off:off + 128],
                                     w_sb[:, kb, :],
                                     start=(kb == 0), stop=(kb == KB - 1))
                ci = nc.vector.tensor_copy(h_sb[:, m, :], h_ps)
                copies.append(ci.ins)
            if hdbg is not None:
                nc.scalar.dma_start(hdbg, h_sb)

            # two triggers on one queue: gathers 0-1 fire once h and their
            # scans are done, so their transfers+writes overlap the
            # remaining prep scans (ring append-while-consume is safe on a
            # single queue; concurrent queues are not).
            trig = nc.gpsimd.trigger_dma(count=None, queue_num=0)
            for ci in copies:
                add_dep_helper(trig.ins, ci, reason="gather reads h")
            add_dep_helper(trig.ins, preps[-1].ins, sync=False,
                           reason="trigger after prep scans")
            views = [r1t.rearrange("(q p) s -> p q s", p=128),
                     r2t.rearrange("(q p) s -> p q s", p=128)]

            def emit_writes(gi, n, off):
                weng = nc.scalar if gi % 2 == 0 else nc.sync
                wg = weng.wait_ge(g_sems[gi], 16)
                add_dep_helper(wg.ins, ild.ins, sync=False,
                               reason="queue issues idx load before blocking")
                add_dep_helper(wg.ins, wld.ins, sync=False,
                               reason="queue issues w load before blocking")
                for xl in xlds:
                    add_dep_helper(wg.ins, xl.ins, sync=False,
                                   reason="queue issues x loads first")
                add_dep_helper(wg.ins, trig.ins, sync=False,
                               reason="wait follows trigger")
                last = None
                lo = off
                while lo < off + n:
                    pair = lo // (2 * cap)
                    hi = min(off + n, (pair + 1) * 2 * cap)
                    last = weng.dma_start(
                        views[pair][:, :, lo - pair * 2 * cap:
                                    hi - pair * 2 * cap],
                        g_tiles[gi][:, :, lo - off:hi - off])
                    add_dep_helper(last.ins, wg.ins,
                                   reason="write after gather DMA sem")
                    lo = hi
                return last

            for gi in range(len(sizes)):
                emit_writes(gi, sizes[gi], offs[gi])

    nc.compile()
    return nc


def _get_nc(cap):
    key = ("nc", cap)
    if key not in _cache:
        _cache[key] = _build_nc(cap)
    return _cache[key]


def _numpy_ref(flag, encoded_input, start_ids_1, end_ids_1, query_batch_idx,
               start_ids_2, end_ids_2, W, b):
    h = encoded_input.astype(np.float32) @ W.astype(np.float32) + \
        b.astype(np.float32)
    qb = np.asarray(query_batch_idx).astype(np.int64)

    def span(s, e):
        s = np.asarray(s).astype(np.int64)
        e = np.asarray(e).astype(np.int64)
        rep = np.concatenate([h[qb, s], h[qb, e]], axis=-1)
        return rep * (e >= s)[:, None].astype(rep.dtype)

    return span(start_ids_1, end_ids_1), span(start_ids_2, end_ids_2)


def _route_pair(s, e, sel, cap):
    """Pack one stream-pair's valid queries (global ids `sel`) into cap
    slots. Returns (idx_s, idx_e, order): cap-long slot->row maps (pads
    point at row 0) and slot->query-id (-1 = pad, dropped by the host)."""
    sv, ev = s[sel], e[sel]
    valid = ev >= sv
    pos = np.nonzero(valid)[0]
    if len(pos) > cap:
        raise ValueError("slot overflow")
    idx_s = np.zeros(cap, np.int64)
    idx_e = np.zeros(cap, np.int64)
    order = np.full(cap, -1, np.int64)
    idx_s[:len(pos)] = sv[pos]
    idx_e[:len(pos)] = ev[pos]
    order[:len(pos)] = sel[pos]
    return idx_s, idx_e, order


def kernel(flag, encoded_input, start_ids_1, end_ids_1, query_batch_idx,
           start_ids_2, end_ids_2, W, b):
    from concourse.bass_utils import run_bass_kernel_spmd

    x_full = np.asarray(encoded_input)
    w_np = np.asarray(W).astype(np.float32)
    b_np = np.asarray(b).astype(np.float32)
    qb = np.asarray(query_batch_idx).astype(np.int64)
    s1 = np.asarray(start_ids_1).astype(np.int64)
    e1 = np.asarray(end_ids_1).astype(np.int64)
    s2 = np.asarray(start_ids_2).astype(np.int64)
    e2 = np.asarray(end_ids_2).astype(np.int64)

    perms = [np.nonzero(qb == bb)[0] for bb in range(B)]
    in_range = (qb.min() >= 0 and qb.max() < B and
                all(a.min() >= 0 and a.max() < S for a in (s1, e1, s2, e2)))
    if not in_range or x_full.shape != (B, S, D):
        res1, res2 = _numpy_ref(flag, x_full, s1, e1, qb, s2, e2, w_np, b_np)
        return np.asarray(res1, np.float32), np.asarray(res2, np.float32)

    # per-(core, pair) valid counts -> slot capacity (64-granular so the
    # combined [s|e] gather length stays a multiple of 128)
    cmax = 1
    for bb in range(B):
        sel = perms[bb]
        for s, e in ((s1, e1), (s2, e2)):
            cmax = max(cmax, int(np.count_nonzero(e[sel] >= s[sel])))
    cap = -(-cmax // 64) * 64

    w16 = np.ascontiguousarray(
        w_np.reshape(KB, 128, P).transpose(1, 0, 2).astype(np.float16))

    in_maps, orders = [], []
    for bb in range(B):
        sel = perms[bb]
        i1s, i1e, order1 = _route_pair(s1, e1, sel, cap)
        i2s, i2e, order2 = _route_pair(s2, e2, sel, cap)
        orders.append((order1, order2))
        idx_flat = np.concatenate([i1s, i1e, i2s, i2e]).astype(np.int16)
        idx_w = idx_flat.reshape(4 * cap // 16, 16).T
        idx_w = np.ascontiguousarray(np.tile(idx_w, (8, 1)))
        # x chunk layout [c, p, k, j]: x16[c, p, k, j] = x[bb][c*CHW+j, k*128+p]
        x16 = np.ascontiguousarray(
            np.asarray(x_full[bb], np.float32)
            .reshape(NCH, CHW, KB, 128).transpose(0, 3, 2, 1)
            .astype(np.float16))
        in_maps.append({
            "x": x16,
            "w": w16,
            "idx": idx_w,
        })

    nc = _get_nc(cap)
    out = run_bass_kernel_spmd(nc, in_maps, core_ids=list(range(NCORES)))
    _cache["last_run"] = out

    bcat = np.concatenate([b_np, b_np]).astype(np.float32)
    res1 = np.zeros((NQ, 2 * P), np.float32)
    res2 = np.zeros((NQ, 2 * P), np.float32)
    for bb in range(B):
        order1, order2 = orders[bb]
        for r, order, res in ((out.results[bb]["r1t"], order1, res1),
                              (out.results[bb]["r2t"], order2, res2)):
            rt = r.T                      # [2*cap, 512]
            real = order >= 0
            vals = np.concatenate([rt[:cap][real], rt[cap:][real]],
                                  axis=1).astype(np.float32)
            res[order[real]] = vals + bcat
    return res1, res2
